# revision 43
# baseline (speedup 1.0000x reference)
"""AttentionTSSA Trainium2 kernel — full-IO contract.

kernel(**inputs) takes the FULL inputs (x [8,512,128,128], qkv_w, temp,
out_w, out_b), shards data-parallel over batch across the 8 NeuronCores
(batch i -> core i), runs a Bass/Tile kernel per core, and returns the
full [8,512,128,128] float32 output.

Layout trick: the qkv OUTPUT channels are permuted on the host so that
in every 128-partition block, partition p belongs to head p//16.  The
head->channel broadcast of Pi then uses a single [8,128] indicator
matmul per token tile (instead of one per channel block), and the same
indicator broadcasts per-head scalars via a 1-column matmul.  The
permutation is inverted for free by permuting out_w's rows.

Softmax linearization: logits = sum_{d in h} w^2/norm2[d] lie in
[0.0014, 0.0078] (they average 64 terms w^2/16384), so with temp=1
  Pi = softmax(l)  ~=  (1+l) * (0.25 - sm/64),   sm = sum_h (1+l)
with relative error < 4e-5 (vs 2e-2 budget).  This removes every
exp/ln/reciprocal from the token loop; both affine steps run on the
scalar engine as Identity activations with scale+bias.

Per-core phases (layout [d, n], d on partitions):
  P1: w = qkv_w @ xb (fp16 matmuls); norm2[d] = sum_n w^2
  P2: sq = w^2 (bf16); lg = lmat^T sq (PE); pu = 1 + temp/256*lg (ACT);
      sm = ones8 @ pu (PE); r = 0.25 - sm/64 (ACT); Pi = pu*r (DVE,
      accum S); pib = ind^T Pi (PE); dots[d] += sum_n sq*pib (DVE)
  P3: attn = 1/(1+dots/(S+eps)) folded into out_w rows; o = w*pib;
      y = (out_w * -attn) @ o + out_b (fp16 matmuls), y stored fp16.

Engine balance per tile (measured costs: ACT ~0.7us, DVE STT ~0.69,
DVE TT 2x ~0.45, GPSIMD TT ~1.37 per [128,512] op):
  P1 PE 3.46 | ACT 2 copies + 2 sq-accum | DVE 2 copies + 2 sq-accum
  P2 PE 1.3  | ACT pu + r + 2 sq | DVE Pi + 4 dots | GPS 2 sq
  P3 PE 3.67 | ACT pib copy + 2 ybias | DVE 2 o + 2 ybias | GPS 2 o
Software pipelining (next tile's squares / o issued ahead) keeps the
PE HAM clock-gate warm (idle > 3.4us halves the PE clock).
"""

import sys

sys.path.insert(0, "/opt/trn_rl_repo")

from contextlib import ExitStack

import numpy as np

import concourse.bass as bass
import concourse.tile as tile
from concourse import bacc, mybir
from concourse.bass_utils import run_bass_kernel_spmd
from concourse.hw_specs import get_activation_tables
F32 = mybir.dt.float32
F16 = mybir.dt.bfloat16   # averaged paths: sq, lmat, scratch
F16A = mybir.dt.float16   # value paths: x, w, Pi, o, weights, y
AF = mybir.ActivationFunctionType
ALU = mybir.AluOpType

B = 8            # batch == number of cores
C = 512          # channels
H_IMG, W_IMG = 128, 128
N = H_IMG * W_IMG
HEADS = 8
HD = 64          # head dim
NT = 512         # tokens per tile
KD = 4           # 128-partition tiles of the channel dim
P = 128
LM_SCALE = 256.0  # keeps invnorm2 out of fp16-subnormal range in lmat

_NC_CACHE = {}


def _dedupe_act_table_loads(nc):
    """Collapse all InstLoadActFuncSet into one load of the set that
    contains every function this kernel uses (square, copy, identity).
    The kernel CFG is a single linear block per engine, so a single
    leading load is sufficient."""
    tables = list(get_activation_tables(nc.m.arch).keys())
    want = {AF.Square, AF.Copy, AF.Identity}
    sets = get_activation_tables(nc.m.arch)
    target = None
    for idx, name in enumerate(tables):
        if want <= sets[name]:
            target = idx
            break
    if target is None:
        return
    first = True
    for blk in nc.main_func.blocks:
        keep = []
        for inst in blk.instructions:
            if isinstance(inst, mybir.InstLoadActFuncSet):
                si = inst.sync_info
                has_sync = si is not None and (
                    len(si.on_wait) > 0 or len(si.on_update) > 0)
                if first or has_sync:
                    inst.act_func_set_id = target
                    first = False
                    keep.append(inst)
            else:
                keep.append(inst)
        blk.instructions[:] = keep


def _build_nc(n_tokens=N, n_cores=B):
    NTILES = n_tokens // NT
    nc = bacc.Bacc("TRN2", target_bir_lowering=False, debug=False,
                   num_devices=n_cores)

    xb = nc.dram_tensor("xb", [C, n_tokens], F16A, kind="ExternalInput").ap()
    qkvwT = nc.dram_tensor("qkvwT", [C, C], F16A, kind="ExternalInput").ap()
    outwT = nc.dram_tensor("outwT", [C, C], F16A, kind="ExternalInput").ap()
    ind = nc.dram_tensor("ind", [HEADS, P], F16A, kind="ExternalInput").ap()
    mask8 = nc.dram_tensor("mask8", [P, HEADS], F32,
                           kind="ExternalInput").ap()
    ones8 = nc.dram_tensor("ones8", [HEADS, HEADS], F16A,
                           kind="ExternalInput").ap()
    temp_s = nc.dram_tensor("temp_s", [HEADS, 1], F32,
                            kind="ExternalInput").ap()
    outb = nc.dram_tensor("outb", [C, 1], F32, kind="ExternalInput").ap()
    y = nc.dram_tensor("y", [C, n_tokens], F16A, kind="ExternalOutput").ap()

    with tile.TileContext(nc) as tc, ExitStack() as top:
        const = top.enter_context(tc.tile_pool(name="const", bufs=1))
        persist = top.enter_context(tc.tile_pool(name="persist", bufs=1))

        # --- constants into SBUF -------------------------------------------
        # qkv weights load first; the rest of the constants are deferred
        # until after the first x tiles are in flight (startup latency)
        qkvwT_sb = [const.tile([P, C], F16A, name=f"qkvwT{k}") for k in range(KD)]
        outwT_sb = [const.tile([P, C], F16A, name=f"outwT{k}") for k in range(KD)]
        for k in range(KD):
            nc.sync.dma_start(qkvwT_sb[k][:], qkvwT[k * P:(k + 1) * P, :])
        ind_sb = const.tile([HEADS, P], F16A, name="ind")
        mask8_sb = const.tile([P, HEADS], F32, name="mask8")
        ones8_sb = const.tile([HEADS, HEADS], F16A, name="ones8")
        temp_sb = const.tile([HEADS, 1], F32, name="temp")
        outb_sb = const.tile([P, KD], F32, name="outb")
        warm_sb = const.tile([P, NT], F16A, name="warmsb")
        nc.vector.memset(warm_sb[:], 0.0)
        bias1 = const.tile([HEADS, 1], F32, name="bias1")
        nc.vector.memset(bias1[:], 1.0)
        biasq = const.tile([HEADS, 1], F32, name="biasq")
        nc.vector.memset(biasq[:], 0.25)

        def load_deferred_consts():
            for k in range(KD):
                nc.sync.dma_start(outwT_sb[k][:], outwT[k * P:(k + 1) * P, :])
            nc.sync.dma_start(ind_sb[:], ind)
            nc.sync.dma_start(mask8_sb[:], mask8)
            nc.sync.dma_start(ones8_sb[:], ones8)
            nc.sync.dma_start(temp_sb[:], temp_s)
            for k in range(KD):
                nc.sync.dma_start(outb_sb[:, k:k + 1],
                                  outb[k * P:(k + 1) * P, :])

        # --- persistent state ----------------------------------------------
        w_store = [persist.tile([P, n_tokens], F16A, name=f"w{k}")
                   for k in range(KD)]
        pi_store = persist.tile([HEADS, n_tokens], F16A, name="pi")
        norm2_part = persist.tile([P, KD * NTILES], F32, name="norm2p")
        dots_part = persist.tile([P, KD * NTILES], F32, name="dotsp")
        s_part = persist.tile([HEADS, NTILES], F32, name="sp")
        inv2 = persist.tile([P, KD], F32, name="inv2")
        lmat = persist.tile([P, KD * HEADS], F16, name="lmat")  # logits lhsT
        nattn = persist.tile([P, KD], F32, name="nattn")
        srb = persist.tile([P, 1], F32, name="srb")
        sv16 = persist.tile([HEADS, 1], F16A, name="sv16")
        outwA = [persist.tile([P, C], F16A, name=f"outwA{k}")
                 for k in range(KD)]

        # =================== Phase 1: qkv matmul + norm2 ===================
        with ExitStack() as p1:
            xpool = p1.enter_context(tc.tile_pool(name="x", bufs=12))
            sqscr = p1.enter_context(tc.tile_pool(name="sqscr", bufs=4))
            wps = p1.enter_context(tc.tile_pool(name="wps", bufs=8, space="PSUM"))

            # prefetch x for the first 2 tiles before anything else queues
            xs_pre = {}
            for t in range(2):
                xs_pre[t] = []
                for kc in range(KD):
                    xt = xpool.tile([P, NT], F16A, tag="x")
                    nc.sync.dma_start(
                        xt[:], xb[kc * P:(kc + 1) * P, t * NT:(t + 1) * NT])
                    xs_pre[t].append(xt)
            load_deferred_consts()

            # HAM warm-up: junk matmuls cover the first x DMA latency so
            # real matmuls start at the 2.4 GHz clock.
            # (warm-up target borrows a wps bank; it rotates back into the
            # accumulation pool after the first few tiles)
            warmpsum = wps.tile([P, NT], F32, tag="wps")
            for _ in range(16):
                nc.tensor.matmul(warmpsum[:], lhsT=warm_sb[:, 0:P],
                                 rhs=warm_sb[:], start=True, stop=True)

            for t in range(NTILES):
                if t in xs_pre:
                    xs = xs_pre.pop(t)
                else:
                    xs = []
                    for kc in range(KD):
                        xt = xpool.tile([P, NT], F16A, tag="x")
                        nc.sync.dma_start(
                            xt[:],
                            xb[kc * P:(kc + 1) * P, t * NT:(t + 1) * NT])
                        xs.append(xt)
                for kd in range(KD):
                    wp = wps.tile([P, NT], F32, tag="wps")
                    for kc in range(KD):
                        nc.tensor.matmul(
                            wp[:],
                            lhsT=qkvwT_sb[kc][:, kd * P:(kd + 1) * P],
                            rhs=xs[kc][:],
                            start=(kc == 0), stop=(kc == KD - 1))
                    w16 = w_store[kd][:, t * NT:(t + 1) * NT]
                    acc = norm2_part[:, kd * NTILES + t:kd * NTILES + t + 1]
                    sq0 = sqscr.tile([P, NT], F16, tag="sqscr")
                    # copy PSUM->SBUF frees the bank; squares read w16
                    if kd < 2:
                        nc.scalar.activation(w16, wp[:], AF.Copy)
                        nc.scalar.activation(sq0[:], w16, AF.Square,
                                             accum_out=acc)
                    else:
                        nc.vector.tensor_copy(w16, wp[:])
                        nc.vector.scalar_tensor_tensor(
                            out=sq0[:], in0=w16, scalar=1.0, in1=w16,
                            op0=ALU.mult, op1=ALU.mult, accum_out=acc)

        # =================== Phase 2: softmax over heads + dots ============
        with ExitStack() as p2:
            sqpool = p2.enter_context(tc.tile_pool(name="sq", bufs=12))
            hpool = p2.enter_context(tc.tile_pool(name="hp", bufs=8))
            scr = p2.enter_context(tc.tile_pool(name="scr", bufs=4))
            lps = p2.enter_context(tc.tile_pool(name="lps", bufs=2, space="PSUM"))
            sps = p2.enter_context(tc.tile_pool(name="sps", bufs=2, space="PSUM"))
            bps = p2.enter_context(tc.tile_pool(name="bps", bufs=3, space="PSUM"))

            def emit_sq_rest(t):
                # squares kd0..2 (bf16): kd0,1 GPSIMD, kd2 ACT
                sqs = []
                for kd in range(3):
                    sq = sqpool.tile([P, NT], F16, tag="sq")
                    w16 = w_store[kd][:, t * NT:(t + 1) * NT]
                    if kd < 2:
                        nc.gpsimd.tensor_tensor(sq[:], w16, w16, op=ALU.mult)
                    else:
                        nc.scalar.activation(sq[:], w16, AF.Square)
                    sqs.append(sq)
                return sqs

            def emit_sq_dve(t):
                # kd3 square on DVE: dependency-free (w16 has been ready
                # since phase 1), issued to fill the DVE's pib-matmul wait
                sq = sqpool.tile([P, NT], F16, tag="sq")
                w16 = w_store[3][:, t * NT:(t + 1) * NT]
                nc.vector.tensor_mul(sq[:], w16, w16)
                return sq

            def emit_sq(t):
                return emit_sq_rest(t) + [emit_sq_dve(t)]

            sq_next = emit_sq(0)

            # --- finalize norm2 -> invnorm2*LM_SCALE -> logits lhsT --------
            for kd in range(KD):
                nc.vector.tensor_reduce(
                    inv2[:, kd:kd + 1],
                    norm2_part[:, kd * NTILES:(kd + 1) * NTILES],
                    axis=mybir.AxisListType.X, op=ALU.add)
            nc.vector.reciprocal(inv2[:], inv2[:])
            nc.vector.tensor_scalar_mul(inv2[:], inv2[:], LM_SCALE)
            for kd in range(KD):
                nc.vector.tensor_scalar(
                    lmat[:, kd * HEADS:(kd + 1) * HEADS], mask8_sb[:],
                    scalar1=inv2[:, kd:kd + 1], scalar2=None, op0=ALU.mult)

            def emit_logits(t, sqs):
                lg = lps.tile([HEADS, NT], F32, tag="lps")
                for i in range(KD):
                    nc.tensor.matmul(
                        lg[:],
                        lhsT=lmat[:, i * HEADS:(i + 1) * HEADS],
                        rhs=sqs[i][:],
                        start=(i == 0), stop=(i == KD - 1))
                return lg

            # software pipeline: while the head-sized ACT->PE->ACT->DVE
            # chain of tile t runs, the PE chews tile t+1's logits matmuls
            # and GPS/ACT compute tile t+2's squares.
            sq_cur = sq_next
            lg_cur = emit_logits(0, sq_cur)
            sq_next = emit_sq(1)
            for t in range(NTILES):
                sqs, lg = sq_cur, lg_cur
                # pu = 1 + temp*logits  (= 1 + (temp/256) * lg)
                pu = hpool.tile([HEADS, NT], F16A, tag="pu")
                nc.scalar.activation(pu[:], lg[:], AF.Identity,
                                     scale=temp_sb[:, 0:1], bias=bias1[:])
                sm = sps.tile([HEADS, NT], F32, tag="sps")
                nc.tensor.matmul(sm[:], lhsT=ones8_sb[:], rhs=pu[:])
                # r = 1/sm ~= 0.25 - sm/64   (sm = 8 + sum_h l, l <= 0.008)
                r16 = hpool.tile([HEADS, NT], F16A, tag="r16")
                nc.scalar.activation(r16[:], sm[:], AF.Identity,
                                     scale=-1.0 / 64.0, bias=biasq[:])
                pi_t = pi_store[:, t * NT:(t + 1) * NT]
                nc.vector.scalar_tensor_tensor(
                    out=pi_t, in0=pu[:], scalar=1.0, in1=r16[:],
                    op0=ALU.mult, op1=ALU.mult,
                    accum_out=s_part[:, t:t + 1])
                # the DVE square of tile t+2 lands right after pi(t) in the
                # DVE queue: it runs during the wait for the pib(t) matmul
                sqd = emit_sq_dve(t + 2) if t + 2 < NTILES else None
                # next tile's logits go to the PE ahead of pib(t): the PE
                # works on them while waiting for pi(t)
                if t + 1 < NTILES:
                    sq_cur = sq_next
                    lg_cur = emit_logits(t + 1, sq_cur)
                pib = bps.tile([P, NT], F32, tag="bps")
                nc.tensor.matmul(pib[:], lhsT=ind_sb[:], rhs=pi_t)
                if t + 2 < NTILES:
                    sq_next = emit_sq_rest(t + 2) + [sqd]
                for kd in range(KD):
                    tscr = scr.tile([P, NT], F16, tag="tscr")
                    nc.vector.scalar_tensor_tensor(
                        out=tscr[:], in0=sqs[kd][:], scalar=1.0,
                        in1=pib[:], op0=ALU.mult, op1=ALU.mult,
                        accum_out=dots_part[:, kd * NTILES + t:
                                            kd * NTILES + t + 1])

            # --- finalize: S, attn, fold -attn into outwT ------------------
            svec = hpool.tile([HEADS, 1], F32, tag="svec")
            nc.vector.tensor_reduce(svec[:], s_part[:],
                                    axis=mybir.AxisListType.X, op=ALU.add)
            nc.vector.tensor_scalar_add(svec[:], svec[:], 1e-8)
            nc.vector.reciprocal(svec[:], svec[:])
            nc.vector.tensor_copy(sv16[:], svec[:])
            # head->channel broadcast of 1/(S+eps) via 1-column matmul
            # (reuses the logits PSUM pool, which is idle by now)
            srb_ps = lps.tile([P, 1], F32, tag="lps")
            nc.tensor.matmul(srb_ps[:], lhsT=ind_sb[:], rhs=sv16[:])
            nc.vector.tensor_copy(srb[:], srb_ps[:])
            for kd in range(KD):
                dk = nattn[:, kd:kd + 1]
                nc.vector.tensor_reduce(
                    dk, dots_part[:, kd * NTILES:(kd + 1) * NTILES],
                    axis=mybir.AxisListType.X, op=ALU.add)
                # dots_n = dots * (1/(S+eps)); attn = 1/(1+dots_n)
                nc.vector.tensor_scalar(
                    dk, dk, scalar1=srb[:], scalar2=1.0,
                    op0=ALU.mult, op1=ALU.add)
                nc.vector.reciprocal(dk, dk)
                nc.vector.tensor_scalar_mul(dk, dk, -1.0)
                nc.vector.tensor_scalar(
                    outwA[kd][:], outwT_sb[kd][:], scalar1=dk,
                    scalar2=None, op0=ALU.mult)

        # =================== Phase 3: o = w*Pi, projection =================
        with ExitStack() as p3:
            opool = p3.enter_context(tc.tile_pool(name="o", bufs=8))
            ypool = p3.enter_context(tc.tile_pool(name="y", bufs=6))
            pib2p = p3.enter_context(tc.tile_pool(name="pib2", bufs=2))
            b2ps = p3.enter_context(tc.tile_pool(name="b2ps", bufs=2, space="PSUM"))
            ops = p3.enter_context(tc.tile_pool(name="ops", bufs=4, space="PSUM"))

            def emit_o(t):
                pib = b2ps.tile([P, NT], F32, tag="b2ps")
                nc.tensor.matmul(
                    pib[:], lhsT=ind_sb[:],
                    rhs=pi_store[:, t * NT:(t + 1) * NT])
                pib_sb = pib2p.tile([P, NT], F16A, tag="pib2")
                nc.scalar.activation(pib_sb[:], pib[:], AF.Copy)
                os_ = []
                for kd in range(KD):
                    ot = opool.tile([P, NT], F16A, tag="o")
                    w16 = w_store[kd][:, t * NT:(t + 1) * NT]
                    if kd < 2:
                        nc.vector.tensor_mul(ot[:], w16, pib_sb[:])
                    else:
                        nc.gpsimd.tensor_mul(ot[:], w16, pib_sb[:])
                    os_.append(ot)
                return os_

            o_next = emit_o(0)
            # re-warm the PE HAM clock gate (phase 2 usually ends cold)
            # while the finalize + o(0) elementwise work runs
            warmpsum3 = ops.tile([P, NT], F32, tag="ops")
            for _ in range(12):
                nc.tensor.matmul(warmpsum3[:], lhsT=warm_sb[:, 0:P],
                                 rhs=warm_sb[:], start=True, stop=True)

            for t in range(NTILES):
                os_ = o_next
                if t + 1 < NTILES:
                    o_next = emit_o(t + 1)
                for kc in range(KD):
                    yp = ops.tile([P, NT], F32, tag="ops")
                    for kd in range(KD):
                        nc.tensor.matmul(
                            yp[:],
                            lhsT=outwA[kd][:, kc * P:(kc + 1) * P],
                            rhs=os_[kd][:],
                            start=(kd == 0), stop=(kd == KD - 1))
                    yt = ypool.tile([P, NT], F16A, tag="y")
                    if kc < 2:
                        nc.scalar.activation(yt[:], yp[:], AF.Identity,
                                             bias=outb_sb[:, kc:kc + 1],
                                             scale=1.0)
                    else:
                        nc.vector.tensor_scalar(
                            yt[:], yp[:], scalar1=outb_sb[:, kc:kc + 1],
                            scalar2=None, op0=ALU.add)
                    nc.sync.dma_start(
                        y[kc * P:(kc + 1) * P, t * NT:(t + 1) * NT], yt[:])

    nc.compile()
    _dedupe_act_table_loads(nc)
    return nc


def _perm():
    """q = kd*128 + p  ->  original channel (p//16)*64 + kd*16 + (p%16).

    Makes head(partition) = p//16 in every 128-channel block."""
    q = np.arange(C)
    kd, p = q // P, q % P
    return (p // 16) * HD + kd * 16 + (p % 16)


def _host_inputs(x, qkv_w, temp, out_w, out_b):
    n_tokens = x.shape[2] * x.shape[3]
    perm = _perm()
    qkvwT = np.ascontiguousarray(np.asarray(qkv_w).T[:, perm]).astype(
        np.float16)
    outwT = np.ascontiguousarray(np.asarray(out_w).T[perm, :]).astype(
        np.float16)
    pp = np.arange(P)
    ind = (pp[None, :] // 16 == np.arange(HEADS)[:, None]).astype(np.float16)
    mask8 = (pp[:, None] // 16 == np.arange(HEADS)[None, :]).astype(
        np.float32)
    ones8 = np.ones((HEADS, HEADS), np.float16)
    temp_sc = (np.asarray(temp, np.float32) / LM_SCALE).reshape(HEADS, 1)
    outb_a = np.asarray(out_b, np.float32).reshape(C, 1)
    maps = []
    for i in range(x.shape[0]):
        maps.append({
            "xb": np.asarray(x[i], np.float32).reshape(C, n_tokens)
            .astype(np.float16),
            "qkvwT": qkvwT, "outwT": outwT, "ind": ind, "mask8": mask8,
            "ones8": ones8, "temp_s": temp_sc, "outb": outb_a,
        })
    return maps


def kernel(x, qkv_w, temp, out_w, out_b):
    x = np.asarray(x)
    b, c, h, w = x.shape
    n_tokens = h * w
    key = (n_tokens, b)
    if key not in _NC_CACHE:
        _NC_CACHE[key] = _build_nc(n_tokens=n_tokens, n_cores=b)
    nc = _NC_CACHE[key]
    in_maps = _host_inputs(x, qkv_w, temp, out_w, out_b)
    res = run_bass_kernel_spmd(nc, in_maps, list(range(b)))
    out = np.stack([np.asarray(res.results[i]["y"], np.float32)
                    .reshape(c, h, w) for i in range(b)])
    return out


# revision 45
# speedup vs baseline: 1.0026x; 1.0026x over previous
"""AttentionTSSA Trainium2 kernel — full-IO contract.

kernel(**inputs) takes the FULL inputs (x [8,512,128,128], qkv_w, temp,
out_w, out_b), shards data-parallel over batch across the 8 NeuronCores
(batch i -> core i), runs a Bass/Tile kernel per core, and returns the
full [8,512,128,128] float32 output.

Layout trick: the qkv OUTPUT channels are permuted on the host so that
in every 128-partition block, partition p belongs to head p//16.  The
head->channel broadcast of Pi then uses a single [8,128] indicator
matmul per token tile (instead of one per channel block), and the same
indicator broadcasts per-head scalars via a 1-column matmul.  The
permutation is inverted for free by permuting out_w's rows.

Softmax linearization: logits = sum_{d in h} w^2/norm2[d] lie in
[0.0014, 0.0078] (they average 64 terms w^2/16384), so with temp=1
  Pi = softmax(l)  ~=  (1+l) * (0.25 - sm/64),   sm = sum_h (1+l)
with relative error < 4e-5 (vs 2e-2 budget).  This removes every
exp/ln/reciprocal from the token loop; both affine steps run on the
scalar engine as Identity activations with scale+bias.

Per-core phases (layout [d, n], d on partitions):
  P1: w = qkv_w @ xb (fp16 matmuls); norm2[d] = sum_n w^2
  P2: sq = w^2 (bf16); lg = lmat^T sq (PE); pu = 1 + temp/256*lg (ACT);
      sm = ones8 @ pu (PE); r = 0.25 - sm/64 (ACT); Pi = pu*r (DVE,
      accum S); pib = ind^T Pi (PE); dots[d] += sum_n sq*pib (DVE)
  P3: attn = 1/(1+dots/(S+eps)) folded into out_w rows; o = w*pib;
      y = (out_w * -attn) @ o + out_b (fp16 matmuls), y stored fp16.

Engine balance per tile (measured costs: ACT ~0.7us, DVE STT ~0.69,
DVE TT 2x ~0.45, GPSIMD TT ~1.37 per [128,512] op):
  P1 PE 3.46 | ACT 2 copies + 2 sq-accum | DVE 2 copies + 2 sq-accum
  P2 PE 1.3  | ACT pu + r + 2 sq | DVE Pi + 4 dots | GPS 2 sq
  P3 PE 3.67 | ACT pib copy + 2 ybias | DVE 2 o + 2 ybias | GPS 2 o
Software pipelining (next tile's squares / o issued ahead) keeps the
PE HAM clock-gate warm (idle > 3.4us halves the PE clock).
"""

import sys

sys.path.insert(0, "/opt/trn_rl_repo")

from contextlib import ExitStack

import numpy as np

import concourse.bass as bass
import concourse.tile as tile
from concourse import bacc, mybir
from concourse.bass_utils import run_bass_kernel_spmd
from concourse.hw_specs import get_activation_tables
F32 = mybir.dt.float32
F16 = mybir.dt.bfloat16   # averaged paths: sq, lmat, scratch
F16A = mybir.dt.float16   # value paths: x, w, Pi, o, weights, y
AF = mybir.ActivationFunctionType
ALU = mybir.AluOpType

B = 8            # batch == number of cores
C = 512          # channels
H_IMG, W_IMG = 128, 128
N = H_IMG * W_IMG
HEADS = 8
HD = 64          # head dim
NT = 512         # tokens per tile
KD = 4           # 128-partition tiles of the channel dim
P = 128
LM_SCALE = 256.0  # keeps invnorm2 out of fp16-subnormal range in lmat

_NC_CACHE = {}


def _dedupe_act_table_loads(nc):
    """Collapse all InstLoadActFuncSet into one load of the set that
    contains every function this kernel uses (square, copy, identity).
    The kernel CFG is a single linear block per engine, so a single
    leading load is sufficient."""
    tables = list(get_activation_tables(nc.m.arch).keys())
    want = {AF.Square, AF.Copy, AF.Identity}
    sets = get_activation_tables(nc.m.arch)
    target = None
    for idx, name in enumerate(tables):
        if want <= sets[name]:
            target = idx
            break
    if target is None:
        return
    first = True
    for blk in nc.main_func.blocks:
        keep = []
        for inst in blk.instructions:
            if isinstance(inst, mybir.InstLoadActFuncSet):
                si = inst.sync_info
                has_sync = si is not None and (
                    len(si.on_wait) > 0 or len(si.on_update) > 0)
                if first or has_sync:
                    inst.act_func_set_id = target
                    first = False
                    keep.append(inst)
            else:
                keep.append(inst)
        blk.instructions[:] = keep


def _build_nc(n_tokens=N, n_cores=B):
    NTILES = n_tokens // NT
    nc = bacc.Bacc("TRN2", target_bir_lowering=False, debug=False,
                   num_devices=n_cores)

    xb = nc.dram_tensor("xb", [C, n_tokens], F16A, kind="ExternalInput").ap()
    qkvwT = nc.dram_tensor("qkvwT", [C, C], F16A, kind="ExternalInput").ap()
    outwT = nc.dram_tensor("outwT", [C, C], F16A, kind="ExternalInput").ap()
    ind = nc.dram_tensor("ind", [HEADS, P], F16A, kind="ExternalInput").ap()
    mask8 = nc.dram_tensor("mask8", [P, HEADS], F32,
                           kind="ExternalInput").ap()
    ones8 = nc.dram_tensor("ones8", [HEADS, HEADS], F16A,
                           kind="ExternalInput").ap()
    temp_s = nc.dram_tensor("temp_s", [HEADS, 1], F32,
                            kind="ExternalInput").ap()
    outb = nc.dram_tensor("outb", [C, 1], F32, kind="ExternalInput").ap()
    y = nc.dram_tensor("y", [C, n_tokens], F16A, kind="ExternalOutput").ap()

    with tile.TileContext(nc) as tc, ExitStack() as top:
        const = top.enter_context(tc.tile_pool(name="const", bufs=1))
        persist = top.enter_context(tc.tile_pool(name="persist", bufs=1))

        # --- constants into SBUF -------------------------------------------
        # qkv weights load first; the rest of the constants are deferred
        # until after the first x tiles are in flight (startup latency)
        qkvwT_sb = [const.tile([P, C], F16A, name=f"qkvwT{k}") for k in range(KD)]
        outwT_sb = [const.tile([P, C], F16A, name=f"outwT{k}") for k in range(KD)]
        for k in range(KD):
            nc.sync.dma_start(qkvwT_sb[k][:], qkvwT[k * P:(k + 1) * P, :])
        ind_sb = const.tile([HEADS, P], F16A, name="ind")
        mask8_sb = const.tile([P, HEADS], F32, name="mask8")
        ones8_sb = const.tile([HEADS, HEADS], F16A, name="ones8")
        temp_sb = const.tile([HEADS, 1], F32, name="temp")
        outb_sb = const.tile([P, KD], F32, name="outb")
        warm_sb = const.tile([P, NT], F16A, name="warmsb")
        nc.vector.memset(warm_sb[:], 0.0)
        bias1 = const.tile([HEADS, 1], F32, name="bias1")
        nc.vector.memset(bias1[:], 1.0)
        biasq = const.tile([HEADS, 1], F32, name="biasq")
        nc.vector.memset(biasq[:], 0.25)

        def load_deferred_consts():
            for k in range(KD):
                nc.sync.dma_start(outwT_sb[k][:], outwT[k * P:(k + 1) * P, :])
            nc.sync.dma_start(ind_sb[:], ind)
            nc.sync.dma_start(mask8_sb[:], mask8)
            nc.sync.dma_start(ones8_sb[:], ones8)
            nc.sync.dma_start(temp_sb[:], temp_s)
            for k in range(KD):
                nc.sync.dma_start(outb_sb[:, k:k + 1],
                                  outb[k * P:(k + 1) * P, :])

        # --- persistent state ----------------------------------------------
        w_store = [persist.tile([P, n_tokens], F16A, name=f"w{k}")
                   for k in range(KD)]
        pi_store = persist.tile([HEADS, n_tokens], F16A, name="pi")
        norm2_part = persist.tile([P, KD * NTILES], F32, name="norm2p")
        dots_part = persist.tile([P, KD * NTILES], F32, name="dotsp")
        s_part = persist.tile([HEADS, NTILES], F32, name="sp")
        inv2 = persist.tile([P, KD], F32, name="inv2")
        lmat = persist.tile([P, KD * HEADS], F16, name="lmat")  # logits lhsT
        nattn = persist.tile([P, KD], F32, name="nattn")
        srb = persist.tile([P, 1], F32, name="srb")
        sv16 = persist.tile([HEADS, 1], F16A, name="sv16")
        outwA = [persist.tile([P, C], F16A, name=f"outwA{k}")
                 for k in range(KD)]

        # =================== Phase 1: qkv matmul + norm2 ===================
        with ExitStack() as p1:
            xpool = p1.enter_context(tc.tile_pool(name="x", bufs=12))
            sqscr = p1.enter_context(tc.tile_pool(name="sqscr", bufs=4))
            wps = p1.enter_context(tc.tile_pool(name="wps", bufs=8, space="PSUM"))

            # prefetch x for the first 2 tiles before anything else queues
            xs_pre = {}
            for t in range(2):
                xs_pre[t] = []
                for kc in range(KD):
                    xt = xpool.tile([P, NT], F16A, tag="x")
                    nc.sync.dma_start(
                        xt[:], xb[kc * P:(kc + 1) * P, t * NT:(t + 1) * NT])
                    xs_pre[t].append(xt)
            load_deferred_consts()

            # HAM warm-up: junk matmuls cover the first x DMA latency so
            # real matmuls start at the 2.4 GHz clock.
            # (warm-up target borrows a wps bank; it rotates back into the
            # accumulation pool after the first few tiles)
            warmpsum = wps.tile([P, NT], F32, tag="wps")
            for _ in range(16):
                nc.tensor.matmul(warmpsum[:], lhsT=warm_sb[:, 0:P],
                                 rhs=warm_sb[:], start=True, stop=True)

            for t in range(NTILES):
                if t in xs_pre:
                    xs = xs_pre.pop(t)
                else:
                    xs = []
                    for kc in range(KD):
                        xt = xpool.tile([P, NT], F16A, tag="x")
                        nc.sync.dma_start(
                            xt[:],
                            xb[kc * P:(kc + 1) * P, t * NT:(t + 1) * NT])
                        xs.append(xt)
                for kd in range(KD):
                    wp = wps.tile([P, NT], F32, tag="wps")
                    for kc in range(KD):
                        nc.tensor.matmul(
                            wp[:],
                            lhsT=qkvwT_sb[kc][:, kd * P:(kd + 1) * P],
                            rhs=xs[kc][:],
                            start=(kc == 0), stop=(kc == KD - 1))
                    w16 = w_store[kd][:, t * NT:(t + 1) * NT]
                    acc = norm2_part[:, kd * NTILES + t:kd * NTILES + t + 1]
                    sq0 = sqscr.tile([P, NT], F16, tag="sqscr")
                    # copy PSUM->SBUF frees the bank; squares read w16
                    if kd < 2:
                        nc.scalar.activation(w16, wp[:], AF.Copy)
                        nc.scalar.activation(sq0[:], w16, AF.Square,
                                             accum_out=acc)
                    else:
                        nc.vector.tensor_copy(w16, wp[:])
                        nc.vector.scalar_tensor_tensor(
                            out=sq0[:], in0=w16, scalar=1.0, in1=w16,
                            op0=ALU.mult, op1=ALU.mult, accum_out=acc)

        # =================== Phase 2: softmax over heads + dots ============
        with ExitStack() as p2:
            sqpool = p2.enter_context(tc.tile_pool(name="sq", bufs=12))
            hpool = p2.enter_context(tc.tile_pool(name="hp", bufs=8))
            scr = p2.enter_context(tc.tile_pool(name="scr", bufs=4))
            lps = p2.enter_context(tc.tile_pool(name="lps", bufs=2, space="PSUM"))
            sps = p2.enter_context(tc.tile_pool(name="sps", bufs=2, space="PSUM"))
            bps = p2.enter_context(tc.tile_pool(name="bps", bufs=3, space="PSUM"))

            def emit_sq(t):
                # 2 GPSIMD + 2 ACT squares (bf16) for the logits rhs / dots
                sqs = []
                for kd in range(KD):
                    sq = sqpool.tile([P, NT], F16, tag="sq")
                    w16 = w_store[kd][:, t * NT:(t + 1) * NT]
                    if kd < 2:
                        nc.gpsimd.tensor_tensor(sq[:], w16, w16, op=ALU.mult)
                    else:
                        nc.scalar.activation(sq[:], w16, AF.Square)
                    sqs.append(sq)
                return sqs

            sq_next = emit_sq(0)

            # --- finalize norm2 -> invnorm2*LM_SCALE -> logits lhsT --------
            for kd in range(KD):
                nc.vector.tensor_reduce(
                    inv2[:, kd:kd + 1],
                    norm2_part[:, kd * NTILES:(kd + 1) * NTILES],
                    axis=mybir.AxisListType.X, op=ALU.add)
            nc.vector.reciprocal(inv2[:], inv2[:])
            nc.vector.tensor_scalar_mul(inv2[:], inv2[:], LM_SCALE)
            for kd in range(KD):
                nc.vector.tensor_scalar(
                    lmat[:, kd * HEADS:(kd + 1) * HEADS], mask8_sb[:],
                    scalar1=inv2[:, kd:kd + 1], scalar2=None, op0=ALU.mult)

            def emit_logits(t, sqs):
                lg = lps.tile([HEADS, NT], F32, tag="lps")
                for i in range(KD):
                    nc.tensor.matmul(
                        lg[:],
                        lhsT=lmat[:, i * HEADS:(i + 1) * HEADS],
                        rhs=sqs[i][:],
                        start=(i == 0), stop=(i == KD - 1))
                return lg

            # software pipeline: while the head-sized ACT->PE->ACT->DVE
            # chain of tile t runs, the PE chews tile t+1's logits matmuls
            # and GPS/ACT compute tile t+2's squares.
            def emit_dots(t, sqs, pib, kds):
                for kd in kds:
                    tscr = scr.tile([P, NT], F16, tag="tscr")
                    nc.vector.scalar_tensor_tensor(
                        out=tscr[:], in0=sqs[kd][:], scalar=1.0,
                        in1=pib[:], op0=ALU.mult, op1=ALU.mult,
                        accum_out=dots_part[:, kd * NTILES + t:
                                            kd * NTILES + t + 1])

            sq_cur = sq_next
            lg_cur = emit_logits(0, sq_cur)
            sq_next = emit_sq(1)
            pend = None  # (t, sqs, pib) with dots kd2,3 still owed
            for t in range(NTILES):
                sqs, lg = sq_cur, lg_cur
                # pu = 1 + temp*logits  (= 1 + (temp/256) * lg)
                pu = hpool.tile([HEADS, NT], F16A, tag="pu")
                nc.scalar.activation(pu[:], lg[:], AF.Identity,
                                     scale=temp_sb[:, 0:1], bias=bias1[:])
                sm = sps.tile([HEADS, NT], F32, tag="sps")
                nc.tensor.matmul(sm[:], lhsT=ones8_sb[:], rhs=pu[:])
                # r = 1/sm ~= 0.25 - sm/64   (sm = 8 + sum_h l, l <= 0.008)
                r16 = hpool.tile([HEADS, NT], F16A, tag="r16")
                nc.scalar.activation(r16[:], sm[:], AF.Identity,
                                     scale=-1.0 / 64.0, bias=biasq[:])
                pi_t = pi_store[:, t * NT:(t + 1) * NT]
                nc.vector.scalar_tensor_tensor(
                    out=pi_t, in0=pu[:], scalar=1.0, in1=r16[:],
                    op0=ALU.mult, op1=ALU.mult,
                    accum_out=s_part[:, t:t + 1])
                # tile t-1's deferred dots fill the DVE's wait for pib(t):
                # their inputs have been ready for a full tile cycle
                if pend is not None:
                    emit_dots(pend[0], pend[1], pend[2], (2, 3))
                # next tile's logits go to the PE ahead of pib(t): the PE
                # works on them while waiting for pi(t)
                if t + 1 < NTILES:
                    sq_cur = sq_next
                    lg_cur = emit_logits(t + 1, sq_cur)
                pib = bps.tile([P, NT], F32, tag="bps")
                nc.tensor.matmul(pib[:], lhsT=ind_sb[:], rhs=pi_t)
                if t + 2 < NTILES:
                    sq_next = emit_sq(t + 2)
                emit_dots(t, sqs, pib, (0, 1))
                pend = (t, sqs, pib)
            emit_dots(pend[0], pend[1], pend[2], (2, 3))

            # --- finalize: S, attn, fold -attn into outwT ------------------
            svec = hpool.tile([HEADS, 1], F32, tag="svec")
            nc.vector.tensor_reduce(svec[:], s_part[:],
                                    axis=mybir.AxisListType.X, op=ALU.add)
            nc.vector.tensor_scalar_add(svec[:], svec[:], 1e-8)
            nc.vector.reciprocal(svec[:], svec[:])
            nc.vector.tensor_copy(sv16[:], svec[:])
            # head->channel broadcast of 1/(S+eps) via 1-column matmul
            # (reuses the logits PSUM pool, which is idle by now)
            srb_ps = lps.tile([P, 1], F32, tag="lps")
            nc.tensor.matmul(srb_ps[:], lhsT=ind_sb[:], rhs=sv16[:])
            nc.vector.tensor_copy(srb[:], srb_ps[:])
            for kd in range(KD):
                dk = nattn[:, kd:kd + 1]
                nc.vector.tensor_reduce(
                    dk, dots_part[:, kd * NTILES:(kd + 1) * NTILES],
                    axis=mybir.AxisListType.X, op=ALU.add)
                # dots_n = dots * (1/(S+eps)); attn = 1/(1+dots_n)
                nc.vector.tensor_scalar(
                    dk, dk, scalar1=srb[:], scalar2=1.0,
                    op0=ALU.mult, op1=ALU.add)
                nc.vector.reciprocal(dk, dk)
                nc.vector.tensor_scalar_mul(dk, dk, -1.0)
                nc.vector.tensor_scalar(
                    outwA[kd][:], outwT_sb[kd][:], scalar1=dk,
                    scalar2=None, op0=ALU.mult)

        # =================== Phase 3: o = w*Pi, projection =================
        with ExitStack() as p3:
            opool = p3.enter_context(tc.tile_pool(name="o", bufs=8))
            ypool = p3.enter_context(tc.tile_pool(name="y", bufs=6))
            pib2p = p3.enter_context(tc.tile_pool(name="pib2", bufs=2))
            b2ps = p3.enter_context(tc.tile_pool(name="b2ps", bufs=2, space="PSUM"))
            ops = p3.enter_context(tc.tile_pool(name="ops", bufs=4, space="PSUM"))

            def emit_o(t):
                pib = b2ps.tile([P, NT], F32, tag="b2ps")
                nc.tensor.matmul(
                    pib[:], lhsT=ind_sb[:],
                    rhs=pi_store[:, t * NT:(t + 1) * NT])
                pib_sb = pib2p.tile([P, NT], F16A, tag="pib2")
                nc.scalar.activation(pib_sb[:], pib[:], AF.Copy)
                os_ = []
                for kd in range(KD):
                    ot = opool.tile([P, NT], F16A, tag="o")
                    w16 = w_store[kd][:, t * NT:(t + 1) * NT]
                    if kd < 2:
                        nc.vector.tensor_mul(ot[:], w16, pib_sb[:])
                    else:
                        nc.gpsimd.tensor_mul(ot[:], w16, pib_sb[:])
                    os_.append(ot)
                return os_

            o_next = emit_o(0)
            # re-warm the PE HAM clock gate (phase 2 usually ends cold)
            # while the finalize + o(0) elementwise work runs
            warmpsum3 = ops.tile([P, NT], F32, tag="ops")
            for _ in range(12):
                nc.tensor.matmul(warmpsum3[:], lhsT=warm_sb[:, 0:P],
                                 rhs=warm_sb[:], start=True, stop=True)

            for t in range(NTILES):
                os_ = o_next
                if t + 1 < NTILES:
                    o_next = emit_o(t + 1)
                for kc in range(KD):
                    yp = ops.tile([P, NT], F32, tag="ops")
                    for kd in range(KD):
                        nc.tensor.matmul(
                            yp[:],
                            lhsT=outwA[kd][:, kc * P:(kc + 1) * P],
                            rhs=os_[kd][:],
                            start=(kd == 0), stop=(kd == KD - 1))
                    yt = ypool.tile([P, NT], F16A, tag="y")
                    if kc < 2:
                        nc.scalar.activation(yt[:], yp[:], AF.Identity,
                                             bias=outb_sb[:, kc:kc + 1],
                                             scale=1.0)
                    else:
                        nc.vector.tensor_scalar(
                            yt[:], yp[:], scalar1=outb_sb[:, kc:kc + 1],
                            scalar2=None, op0=ALU.add)
                    nc.sync.dma_start(
                        y[kc * P:(kc + 1) * P, t * NT:(t + 1) * NT], yt[:])

    nc.compile()
    _dedupe_act_table_loads(nc)
    return nc


def _perm():
    """q = kd*128 + p  ->  original channel (p//16)*64 + kd*16 + (p%16).

    Makes head(partition) = p//16 in every 128-channel block."""
    q = np.arange(C)
    kd, p = q // P, q % P
    return (p // 16) * HD + kd * 16 + (p % 16)


def _host_inputs(x, qkv_w, temp, out_w, out_b):
    n_tokens = x.shape[2] * x.shape[3]
    perm = _perm()
    qkvwT = np.ascontiguousarray(np.asarray(qkv_w).T[:, perm]).astype(
        np.float16)
    outwT = np.ascontiguousarray(np.asarray(out_w).T[perm, :]).astype(
        np.float16)
    pp = np.arange(P)
    ind = (pp[None, :] // 16 == np.arange(HEADS)[:, None]).astype(np.float16)
    mask8 = (pp[:, None] // 16 == np.arange(HEADS)[None, :]).astype(
        np.float32)
    ones8 = np.ones((HEADS, HEADS), np.float16)
    temp_sc = (np.asarray(temp, np.float32) / LM_SCALE).reshape(HEADS, 1)
    outb_a = np.asarray(out_b, np.float32).reshape(C, 1)
    maps = []
    for i in range(x.shape[0]):
        maps.append({
            "xb": np.asarray(x[i], np.float32).reshape(C, n_tokens)
            .astype(np.float16),
            "qkvwT": qkvwT, "outwT": outwT, "ind": ind, "mask8": mask8,
            "ones8": ones8, "temp_s": temp_sc, "outb": outb_a,
        })
    return maps


def kernel(x, qkv_w, temp, out_w, out_b):
    x = np.asarray(x)
    b, c, h, w = x.shape
    n_tokens = h * w
    key = (n_tokens, b)
    if key not in _NC_CACHE:
        _NC_CACHE[key] = _build_nc(n_tokens=n_tokens, n_cores=b)
    nc = _NC_CACHE[key]
    in_maps = _host_inputs(x, qkv_w, temp, out_w, out_b)
    res = run_bass_kernel_spmd(nc, in_maps, list(range(b)))
    out = np.stack([np.asarray(res.results[i]["y"], np.float32)
                    .reshape(c, h, w) for i in range(b)])
    return out


# revision 50
# speedup vs baseline: 1.0043x; 1.0017x over previous
"""AttentionTSSA Trainium2 kernel — full-IO contract.

kernel(**inputs) takes the FULL inputs (x [8,512,128,128], qkv_w, temp,
out_w, out_b), shards data-parallel over batch across the 8 NeuronCores
(batch i -> core i), runs a Bass/Tile kernel per core, and returns the
full [8,512,128,128] float32 output.

Layout trick: the qkv OUTPUT channels are permuted on the host so that
in every 128-partition block, partition p belongs to head p//16.  The
head->channel broadcast of Pi then uses a single [8,128] indicator
matmul per token tile (instead of one per channel block), and the same
indicator broadcasts per-head scalars via a 1-column matmul.  The
permutation is inverted for free by permuting out_w's rows.

Softmax linearization: logits = sum_{d in h} w^2/norm2[d] lie in
[0.0014, 0.0078] (they average 64 terms w^2/16384), so with temp=1
  Pi = softmax(l)  ~=  (1+l) * (0.25 - sm/64),   sm = sum_h (1+l)
with relative error < 4e-5 (vs 2e-2 budget).  This removes every
exp/ln/reciprocal from the token loop; both affine steps run on the
scalar engine as Identity activations with scale+bias.

Per-core phases (layout [d, n], d on partitions):
  P1: w = qkv_w @ xb (fp16 matmuls); norm2[d] = sum_n w^2
  P2: sq = w^2 (bf16); lg = lmat^T sq (PE); pu = 1 + temp/256*lg (ACT);
      sm = ones8 @ pu (PE); r = 0.25 - sm/64 (ACT); Pi = pu*r (DVE,
      accum S); pib = ind^T Pi (PE); dots[d] += sum_n sq*pib (DVE)
  P3: attn = 1/(1+dots/(S+eps)) folded into out_w rows; o = w*pib;
      y = (out_w * -attn) @ o + out_b (fp16 matmuls), y stored fp16.

Engine balance per tile (measured costs: ACT ~0.7us, DVE STT ~0.69,
DVE TT 2x ~0.45, GPSIMD TT ~1.37 per [128,512] op):
  P1 PE 3.46 | ACT 2 copies + 2 sq-accum | DVE 2 copies + 2 sq-accum
  P2 PE 1.3  | ACT pu + r + 2 sq | DVE Pi + 4 dots | GPS 2 sq
  P3 PE 3.67 | ACT pib copy + 2 ybias | DVE 2 o + 2 ybias | GPS 2 o
Software pipelining (next tile's squares / o issued ahead) keeps the
PE HAM clock-gate warm (idle > 3.4us halves the PE clock).
"""

import sys

sys.path.insert(0, "/opt/trn_rl_repo")

from contextlib import ExitStack

import numpy as np

import concourse.bass as bass
import concourse.tile as tile
from concourse import bacc, mybir
from concourse.bass_utils import run_bass_kernel_spmd
from concourse.hw_specs import get_activation_tables
F32 = mybir.dt.float32
F16 = mybir.dt.bfloat16   # averaged paths: sq, lmat, scratch
F16A = mybir.dt.float16   # value paths: x, w, Pi, o, weights, y
AF = mybir.ActivationFunctionType
ALU = mybir.AluOpType

B = 8            # batch == number of cores
C = 512          # channels
H_IMG, W_IMG = 128, 128
N = H_IMG * W_IMG
HEADS = 8
HD = 64          # head dim
NT = 512         # tokens per tile
KD = 4           # 128-partition tiles of the channel dim
P = 128
LM_SCALE = 256.0  # keeps invnorm2 out of fp16-subnormal range in lmat

_NC_CACHE = {}


def _dedupe_act_table_loads(nc):
    """Collapse all InstLoadActFuncSet into one load of the set that
    contains every function this kernel uses (square, copy, identity).
    The kernel CFG is a single linear block per engine, so a single
    leading load is sufficient."""
    tables = list(get_activation_tables(nc.m.arch).keys())
    want = {AF.Square, AF.Copy, AF.Identity}
    sets = get_activation_tables(nc.m.arch)
    target = None
    for idx, name in enumerate(tables):
        if want <= sets[name]:
            target = idx
            break
    if target is None:
        return
    first = True
    for blk in nc.main_func.blocks:
        keep = []
        for inst in blk.instructions:
            if isinstance(inst, mybir.InstLoadActFuncSet):
                si = inst.sync_info
                has_sync = si is not None and (
                    len(si.on_wait) > 0 or len(si.on_update) > 0)
                if first or has_sync:
                    inst.act_func_set_id = target
                    first = False
                    keep.append(inst)
            else:
                keep.append(inst)
        blk.instructions[:] = keep


def _build_nc(n_tokens=N, n_cores=B):
    NTILES = n_tokens // NT
    nc = bacc.Bacc("TRN2", target_bir_lowering=False, debug=False,
                   num_devices=n_cores)

    xb = nc.dram_tensor("xb", [C, n_tokens], F16A, kind="ExternalInput").ap()
    qkvwT = nc.dram_tensor("qkvwT", [C, C], F16A, kind="ExternalInput").ap()
    outwT = nc.dram_tensor("outwT", [C, C], F16A, kind="ExternalInput").ap()
    ind = nc.dram_tensor("ind", [HEADS, P], F16A, kind="ExternalInput").ap()
    mask8 = nc.dram_tensor("mask8", [P, HEADS], F32,
                           kind="ExternalInput").ap()
    ones8 = nc.dram_tensor("ones8", [HEADS, HEADS], F16A,
                           kind="ExternalInput").ap()
    temp_s = nc.dram_tensor("temp_s", [HEADS, 1], F32,
                            kind="ExternalInput").ap()
    outb = nc.dram_tensor("outb", [C, 1], F32, kind="ExternalInput").ap()
    y = nc.dram_tensor("y", [C, n_tokens], F16A, kind="ExternalOutput").ap()

    with tile.TileContext(nc) as tc, ExitStack() as top:
        const = top.enter_context(tc.tile_pool(name="const", bufs=1))
        persist = top.enter_context(tc.tile_pool(name="persist", bufs=1))

        # --- constants into SBUF -------------------------------------------
        # qkv weights load first; the rest of the constants are deferred
        # until after the first x tiles are in flight (startup latency)
        qkvwT_sb = [const.tile([P, C], F16A, name=f"qkvwT{k}") for k in range(KD)]
        outwT_sb = [const.tile([P, C], F16A, name=f"outwT{k}") for k in range(KD)]
        for k in range(KD):
            nc.sync.dma_start(qkvwT_sb[k][:], qkvwT[k * P:(k + 1) * P, :])
        ind_sb = const.tile([HEADS, P], F16A, name="ind")
        mask8_sb = const.tile([P, HEADS], F32, name="mask8")
        ones8_sb = const.tile([HEADS, HEADS], F16A, name="ones8")
        temp_sb = const.tile([HEADS, 1], F32, name="temp")
        outb_sb = const.tile([P, KD], F32, name="outb")
        warm_sb = const.tile([P, NT], F16A, name="warmsb")
        nc.vector.memset(warm_sb[:], 0.0)
        bias1 = const.tile([HEADS, 1], F32, name="bias1")
        nc.vector.memset(bias1[:], 1.0)
        biasq = const.tile([HEADS, 1], F32, name="biasq")
        nc.vector.memset(biasq[:], 0.25)

        def load_deferred_consts():
            for k in range(KD):
                nc.sync.dma_start(outwT_sb[k][:], outwT[k * P:(k + 1) * P, :])
            nc.sync.dma_start(ind_sb[:], ind)
            nc.sync.dma_start(mask8_sb[:], mask8)
            nc.sync.dma_start(ones8_sb[:], ones8)
            nc.sync.dma_start(temp_sb[:], temp_s)
            for k in range(KD):
                nc.sync.dma_start(outb_sb[:, k:k + 1],
                                  outb[k * P:(k + 1) * P, :])

        # --- persistent state ----------------------------------------------
        w_store = [persist.tile([P, n_tokens], F16A, name=f"w{k}")
                   for k in range(KD)]
        pi_store = persist.tile([HEADS, n_tokens], F16A, name="pi")
        norm2_part = persist.tile([P, KD * NTILES], F32, name="norm2p")
        dots_part = persist.tile([P, KD * NTILES], F32, name="dotsp")
        s_part = persist.tile([HEADS, NTILES], F32, name="sp")
        inv2 = persist.tile([P, KD], F32, name="inv2")
        lmat = persist.tile([P, KD * HEADS], F16, name="lmat")  # logits lhsT
        nattn = persist.tile([P, KD], F32, name="nattn")
        srb = persist.tile([P, 1], F32, name="srb")
        sv16 = persist.tile([HEADS, 1], F16A, name="sv16")
        outwA = [persist.tile([P, C], F16A, name=f"outwA{k}")
                 for k in range(KD)]

        # =================== Phase 1: qkv matmul + norm2 ===================
        with ExitStack() as p1:
            xpool = p1.enter_context(tc.tile_pool(name="x", bufs=6))
            sqscr = p1.enter_context(tc.tile_pool(name="sqscr", bufs=4))
            wps = p1.enter_context(tc.tile_pool(name="wps", bufs=8, space="PSUM"))

            def load_x(t):
                # one DMA per PAIR of channel blocks: [256, NT] of DRAM ->
                # [128, 2*NT] SBUF (row r -> partition r%128, half r//128),
                # halving sync-engine descriptor work
                pair = []
                for k2 in range(KD // 2):
                    xt = xpool.tile([P, 2 * NT], F16A, tag="x")
                    src = (xb[k2 * 2 * P:(k2 + 1) * 2 * P,
                              t * NT:(t + 1) * NT]
                           .rearrange("(two p) n -> p two n", two=2))
                    nc.sync.dma_start(
                        xt[:].rearrange("p (two n) -> p two n", two=2), src)
                    pair.append(xt)
                return [pair[kc // 2][:, (kc % 2) * NT:(kc % 2 + 1) * NT]
                        for kc in range(KD)]

            # prefetch x for the first 2 tiles before anything else queues
            xs_pre = {t: load_x(t) for t in range(2)}
            load_deferred_consts()

            # HAM warm-up: junk matmuls cover the first x DMA latency so
            # real matmuls start at the 2.4 GHz clock.
            # (warm-up target borrows a wps bank; it rotates back into the
            # accumulation pool after the first few tiles)
            warmpsum = wps.tile([P, NT], F32, tag="wps")
            for _ in range(16):
                nc.tensor.matmul(warmpsum[:], lhsT=warm_sb[:, 0:P],
                                 rhs=warm_sb[:], start=True, stop=True)

            for t in range(NTILES):
                xs = xs_pre.pop(t) if t in xs_pre else load_x(t)
                for kd in range(KD):
                    wp = wps.tile([P, NT], F32, tag="wps")
                    for kc in range(KD):
                        nc.tensor.matmul(
                            wp[:],
                            lhsT=qkvwT_sb[kc][:, kd * P:(kd + 1) * P],
                            rhs=xs[kc][:],
                            start=(kc == 0), stop=(kc == KD - 1))
                    w16 = w_store[kd][:, t * NT:(t + 1) * NT]
                    acc = norm2_part[:, kd * NTILES + t:kd * NTILES + t + 1]
                    sq0 = sqscr.tile([P, NT], F16, tag="sqscr")
                    # copy PSUM->SBUF frees the bank; squares read w16
                    if kd < 2:
                        nc.scalar.activation(w16, wp[:], AF.Copy)
                        nc.scalar.activation(sq0[:], w16, AF.Square,
                                             accum_out=acc)
                    else:
                        nc.vector.tensor_copy(w16, wp[:])
                        nc.vector.scalar_tensor_tensor(
                            out=sq0[:], in0=w16, scalar=1.0, in1=w16,
                            op0=ALU.mult, op1=ALU.mult, accum_out=acc)

        # =================== Phase 2: softmax over heads + dots ============
        with ExitStack() as p2:
            sqpool = p2.enter_context(tc.tile_pool(name="sq", bufs=12))
            hpool = p2.enter_context(tc.tile_pool(name="hp", bufs=8))
            scr = p2.enter_context(tc.tile_pool(name="scr", bufs=4))
            lps = p2.enter_context(tc.tile_pool(name="lps", bufs=2, space="PSUM"))
            sps = p2.enter_context(tc.tile_pool(name="sps", bufs=2, space="PSUM"))
            bps = p2.enter_context(tc.tile_pool(name="bps", bufs=3, space="PSUM"))

            def emit_sq(t):
                # 2 GPSIMD + 2 ACT squares (bf16) for the logits rhs / dots
                sqs = []
                for kd in range(KD):
                    sq = sqpool.tile([P, NT], F16, tag="sq")
                    w16 = w_store[kd][:, t * NT:(t + 1) * NT]
                    if kd < 2:
                        nc.gpsimd.tensor_tensor(sq[:], w16, w16, op=ALU.mult)
                    else:
                        nc.scalar.activation(sq[:], w16, AF.Square)
                    sqs.append(sq)
                return sqs

            sq_next = emit_sq(0)

            # --- finalize norm2 -> invnorm2*LM_SCALE -> logits lhsT --------
            for kd in range(KD):
                nc.vector.tensor_reduce(
                    inv2[:, kd:kd + 1],
                    norm2_part[:, kd * NTILES:(kd + 1) * NTILES],
                    axis=mybir.AxisListType.X, op=ALU.add)
            nc.vector.reciprocal(inv2[:], inv2[:])
            nc.vector.tensor_scalar_mul(inv2[:], inv2[:], LM_SCALE)
            for kd in range(KD):
                nc.vector.tensor_scalar(
                    lmat[:, kd * HEADS:(kd + 1) * HEADS], mask8_sb[:],
                    scalar1=inv2[:, kd:kd + 1], scalar2=None, op0=ALU.mult)

            def emit_logits(t, sqs):
                lg = lps.tile([HEADS, NT], F32, tag="lps")
                for i in range(KD):
                    nc.tensor.matmul(
                        lg[:],
                        lhsT=lmat[:, i * HEADS:(i + 1) * HEADS],
                        rhs=sqs[i][:],
                        start=(i == 0), stop=(i == KD - 1))
                return lg

            # software pipeline: while the head-sized ACT->PE->ACT->DVE
            # chain of tile t runs, the PE chews tile t+1's logits matmuls
            # and GPS/ACT compute tile t+2's squares.
            def emit_dots(t, sqs, pib, kds):
                for kd in kds:
                    tscr = scr.tile([P, NT], F16, tag="tscr")
                    nc.vector.scalar_tensor_tensor(
                        out=tscr[:], in0=sqs[kd][:], scalar=1.0,
                        in1=pib[:], op0=ALU.mult, op1=ALU.mult,
                        accum_out=dots_part[:, kd * NTILES + t:
                                            kd * NTILES + t + 1])

            sq_cur = sq_next
            lg_cur = emit_logits(0, sq_cur)
            sq_next = emit_sq(1)
            pend = None  # (t, sqs, pib) with dots kd2,3 still owed
            for t in range(NTILES):
                sqs, lg = sq_cur, lg_cur
                # pu = 1 + temp*logits  (= 1 + (temp/256) * lg)
                pu = hpool.tile([HEADS, NT], F16A, tag="pu")
                nc.scalar.activation(pu[:], lg[:], AF.Identity,
                                     scale=temp_sb[:, 0:1], bias=bias1[:])
                sm = sps.tile([HEADS, NT], F32, tag="sps")
                nc.tensor.matmul(sm[:], lhsT=ones8_sb[:], rhs=pu[:])
                # r = 1/sm ~= 0.25 - sm/64   (sm = 8 + sum_h l, l <= 0.008)
                r16 = hpool.tile([HEADS, NT], F16A, tag="r16")
                nc.scalar.activation(r16[:], sm[:], AF.Identity,
                                     scale=-1.0 / 64.0, bias=biasq[:])
                pi_t = pi_store[:, t * NT:(t + 1) * NT]
                nc.vector.scalar_tensor_tensor(
                    out=pi_t, in0=pu[:], scalar=1.0, in1=r16[:],
                    op0=ALU.mult, op1=ALU.mult,
                    accum_out=s_part[:, t:t + 1])
                # tile t-1's deferred dots fill the DVE's wait for pib(t):
                # their inputs have been ready for a full tile cycle
                if pend is not None:
                    emit_dots(pend[0], pend[1], pend[2], (2, 3))
                # next tile's logits go to the PE ahead of pib(t): the PE
                # works on them while waiting for pi(t)
                if t + 1 < NTILES:
                    sq_cur = sq_next
                    lg_cur = emit_logits(t + 1, sq_cur)
                pib = bps.tile([P, NT], F32, tag="bps")
                nc.tensor.matmul(pib[:], lhsT=ind_sb[:], rhs=pi_t)
                if t + 2 < NTILES:
                    sq_next = emit_sq(t + 2)
                emit_dots(t, sqs, pib, (0, 1))
                pend = (t, sqs, pib)
            emit_dots(pend[0], pend[1], pend[2], (2, 3))

            # --- finalize: S, attn, fold -attn into outwT ------------------
            svec = hpool.tile([HEADS, 1], F32, tag="svec")
            nc.vector.tensor_reduce(svec[:], s_part[:],
                                    axis=mybir.AxisListType.X, op=ALU.add)
            nc.vector.tensor_scalar_add(svec[:], svec[:], 1e-8)
            nc.vector.reciprocal(svec[:], svec[:])
            nc.vector.tensor_copy(sv16[:], svec[:])
            # head->channel broadcast of 1/(S+eps) via 1-column matmul
            # (reuses the logits PSUM pool, which is idle by now)
            srb_ps = lps.tile([P, 1], F32, tag="lps")
            nc.tensor.matmul(srb_ps[:], lhsT=ind_sb[:], rhs=sv16[:])
            nc.vector.tensor_copy(srb[:], srb_ps[:])
            for kd in range(KD):
                dk = nattn[:, kd:kd + 1]
                nc.vector.tensor_reduce(
                    dk, dots_part[:, kd * NTILES:(kd + 1) * NTILES],
                    axis=mybir.AxisListType.X, op=ALU.add)
                # dots_n = dots * (1/(S+eps)); attn = 1/(1+dots_n)
                nc.vector.tensor_scalar(
                    dk, dk, scalar1=srb[:], scalar2=1.0,
                    op0=ALU.mult, op1=ALU.add)
                nc.vector.reciprocal(dk, dk)
                nc.vector.tensor_scalar_mul(dk, dk, -1.0)
                nc.vector.tensor_scalar(
                    outwA[kd][:], outwT_sb[kd][:], scalar1=dk,
                    scalar2=None, op0=ALU.mult)

        # =================== Phase 3: o = w*Pi, projection =================
        with ExitStack() as p3:
            opool = p3.enter_context(tc.tile_pool(name="o", bufs=8))
            ypool = p3.enter_context(tc.tile_pool(name="y", bufs=4))
            pib2p = p3.enter_context(tc.tile_pool(name="pib2", bufs=2))
            b2ps = p3.enter_context(tc.tile_pool(name="b2ps", bufs=2, space="PSUM"))
            ops = p3.enter_context(tc.tile_pool(name="ops", bufs=4, space="PSUM"))

            def emit_o(t):
                pib = b2ps.tile([P, NT], F32, tag="b2ps")
                nc.tensor.matmul(
                    pib[:], lhsT=ind_sb[:],
                    rhs=pi_store[:, t * NT:(t + 1) * NT])
                pib_sb = pib2p.tile([P, NT], F16A, tag="pib2")
                nc.scalar.activation(pib_sb[:], pib[:], AF.Copy)
                os_ = []
                for kd in range(KD):
                    ot = opool.tile([P, NT], F16A, tag="o")
                    w16 = w_store[kd][:, t * NT:(t + 1) * NT]
                    if kd < 2:
                        nc.vector.tensor_mul(ot[:], w16, pib_sb[:])
                    else:
                        nc.gpsimd.tensor_mul(ot[:], w16, pib_sb[:])
                    os_.append(ot)
                return os_

            o_next = emit_o(0)
            # re-warm the PE HAM clock gate (phase 2 usually ends cold)
            # while the finalize + o(0) elementwise work runs
            warmpsum3 = ops.tile([P, NT], F32, tag="ops")
            for _ in range(12):
                nc.tensor.matmul(warmpsum3[:], lhsT=warm_sb[:, 0:P],
                                 rhs=warm_sb[:], start=True, stop=True)

            for t in range(NTILES):
                os_ = o_next
                if t + 1 < NTILES:
                    o_next = emit_o(t + 1)
                yt_pair = None
                for kc in range(KD):
                    yp = ops.tile([P, NT], F32, tag="ops")
                    for kd in range(KD):
                        nc.tensor.matmul(
                            yp[:],
                            lhsT=outwA[kd][:, kc * P:(kc + 1) * P],
                            rhs=os_[kd][:],
                            start=(kd == 0), stop=(kd == KD - 1))
                    if kc % 2 == 0:
                        yt_pair = ypool.tile([P, 2 * NT], F16A, tag="y")
                    yt = yt_pair[:, (kc % 2) * NT:(kc % 2 + 1) * NT]
                    if kc < 2:
                        nc.scalar.activation(yt, yp[:], AF.Identity,
                                             bias=outb_sb[:, kc:kc + 1],
                                             scale=1.0)
                    else:
                        nc.vector.tensor_scalar(
                            yt, yp[:], scalar1=outb_sb[:, kc:kc + 1],
                            scalar2=None, op0=ALU.add)
                    if kc % 2 == 1:
                        # one DMA per pair of channel blocks: [128, 2*NT]
                        # SBUF -> [256, NT] of DRAM
                        k2 = kc // 2
                        dst = (y[k2 * 2 * P:(k2 + 1) * 2 * P,
                                 t * NT:(t + 1) * NT]
                               .rearrange("(two p) n -> p two n", two=2))
                        nc.sync.dma_start(
                            dst,
                            yt_pair[:].rearrange("p (two n) -> p two n",
                                                 two=2))

    nc.compile()
    _dedupe_act_table_loads(nc)
    return nc


def _perm():
    """q = kd*128 + p  ->  original channel (p//16)*64 + kd*16 + (p%16).

    Makes head(partition) = p//16 in every 128-channel block."""
    q = np.arange(C)
    kd, p = q // P, q % P
    return (p // 16) * HD + kd * 16 + (p % 16)


def _host_inputs(x, qkv_w, temp, out_w, out_b):
    n_tokens = x.shape[2] * x.shape[3]
    perm = _perm()
    qkvwT = np.ascontiguousarray(np.asarray(qkv_w).T[:, perm]).astype(
        np.float16)
    outwT = np.ascontiguousarray(np.asarray(out_w).T[perm, :]).astype(
        np.float16)
    pp = np.arange(P)
    ind = (pp[None, :] // 16 == np.arange(HEADS)[:, None]).astype(np.float16)
    mask8 = (pp[:, None] // 16 == np.arange(HEADS)[None, :]).astype(
        np.float32)
    ones8 = np.ones((HEADS, HEADS), np.float16)
    temp_sc = (np.asarray(temp, np.float32) / LM_SCALE).reshape(HEADS, 1)
    outb_a = np.asarray(out_b, np.float32).reshape(C, 1)
    maps = []
    for i in range(x.shape[0]):
        maps.append({
            "xb": np.asarray(x[i], np.float32).reshape(C, n_tokens)
            .astype(np.float16),
            "qkvwT": qkvwT, "outwT": outwT, "ind": ind, "mask8": mask8,
            "ones8": ones8, "temp_s": temp_sc, "outb": outb_a,
        })
    return maps


def kernel(x, qkv_w, temp, out_w, out_b):
    x = np.asarray(x)
    b, c, h, w = x.shape
    n_tokens = h * w
    key = (n_tokens, b)
    if key not in _NC_CACHE:
        _NC_CACHE[key] = _build_nc(n_tokens=n_tokens, n_cores=b)
    nc = _NC_CACHE[key]
    in_maps = _host_inputs(x, qkv_w, temp, out_w, out_b)
    res = run_bass_kernel_spmd(nc, in_maps, list(range(b)))
    out = np.stack([np.asarray(res.results[i]["y"], np.float32)
                    .reshape(c, h, w) for i in range(b)])
    return out
